# revision 3
# baseline (speedup 1.0000x reference)
"""Per-channel Linear(seq->pred) over channels, 8-core channel-parallel Trainium2 kernel.

Math: y[b,p,c] = sum_s x[b,s,c] * W[c,p,s] + bias[c,p]

Strategy:
  - Shard channels C=321 across 8 cores (pad to 328 = 8*41).
  - Host-side re-layout:
      wt[c,s,p] = W[c,p,s], with an extra row wt[c,720,p] = bias[c,p]
      xt[c,s,b] = x[b,s,c], with an extra row xt[c,720,b] = 1.0
    so bias is folded into the contraction (K = 721).
  - Per channel: Y_c[b,p] = sum_k xT_chunk[k].T @ wT_chunk[k], accumulated in
    PSUM over 6 K-chunks (5x128 + 81). lhsT = xT chunk [K,64] (stationary),
    rhs = wT chunk [K,720] streamed as N = 512 + 208 (PSUM bank limit).
  - Two channels share one PSUM tile via PE column tiling: channel A in
    output partitions 0:64, channel B in 64:128 (concurrent col-groups).
  - Result copied PSUM->SBUF (DVE + ACT split) and DMA'd out as y[c,b,p].
"""

import numpy as np

import concourse.bacc as bacc
import concourse.mybir as mybir
import concourse.tile as tile
from concourse.bass_utils import run_bass_kernel_spmd

F32 = mybir.dt.float32

B = 64          # batch
S = 720         # seq_len (contraction)
P = 720         # pred_len
C = 321         # channels
N_CORES = 8
CL = 41         # channels per core; 8*41 = 328 >= 321
CPAD = N_CORES * CL
SP1 = S + 1     # contraction rows incl. bias row
KFULL = 5       # number of full 128-row K chunks
KTAIL = SP1 - KFULL * 128  # 81
NSPLIT = 512    # first matmul N (PSUM bank holds 512 f32)

_CACHE: dict = {}


def _build_module():
    nc = bacc.Bacc("TRN2", target_bir_lowering=False, debug=False,
                   num_devices=N_CORES)
    wt = nc.dram_tensor("wt", [CL, SP1, P], F32, kind="ExternalInput").ap()
    xt = nc.dram_tensor("xt", [CL, SP1, B], F32, kind="ExternalInput").ap()
    y = nc.dram_tensor("y", [CL, B, P], F32, kind="ExternalOutput").ap()

    with tile.TileContext(nc) as tc:
        with (
            tc.tile_pool(name="wp", bufs=3) as wp,
            tc.tile_pool(name="wtp", bufs=3) as wtp,
            tc.tile_pool(name="xp", bufs=3) as xp,
            tc.tile_pool(name="xtp", bufs=3) as xtp,
            tc.tile_pool(name="pp", bufs=3, space="PSUM") as pp,
            tc.tile_pool(name="op", bufs=3) as op,
        ):
            def load_channel(c):
                wbig = wp.tile([128, KFULL, P], F32, name=f"wbig{c}", tag="wbig")
                wlast = wtp.tile([128, P], F32, name=f"wlast{c}", tag="wlast")
                xbig = xp.tile([128, KFULL, B], F32, name=f"xbig{c}", tag="xbig")
                xlast = xtp.tile([128, B], F32, name=f"xlast{c}", tag="xlast")
                nc.sync.dma_start(
                    wbig[:], wt[c, 0:KFULL * 128, :].rearrange("(k s) p -> s k p", s=128))
                nc.sync.dma_start(wlast[:KTAIL, :], wt[c, KFULL * 128:SP1, :])
                nc.scalar.dma_start(
                    xbig[:], xt[c, 0:KFULL * 128, :].rearrange("(k s) b -> s k b", s=128))
                nc.scalar.dma_start(xlast[:KTAIL, :], xt[c, KFULL * 128:SP1, :])
                return wbig, wlast, xbig, xlast

            def mm_operands(tiles, k):
                wbig, wlast, xbig, xlast = tiles
                if k < KFULL:
                    return xbig[:, k, :], wbig[:, k, 0:NSPLIT], wbig[:, k, NSPLIT:P]
                return (xlast[:KTAIL, :], wlast[:KTAIL, 0:NSPLIT],
                        wlast[:KTAIL, NSPLIT:P])

            # process channels in pairs: two channels share one PSUM tile
            # (output partitions 0:64 and 64:128 -> PE column tiling).
            # Matmuls of the two channels are interleaved so the two
            # 64-wide column groups stream concurrently in the PE array.
            for c0 in range(0, CL, 2):
                pair = min(2, CL - c0)
                tiles = [load_channel(c0)]
                if pair == 2:
                    tiles.append(load_channel(c0 + 1))
                ps = pp.tile([pair * B, P], F32, name=f"ps{c0}", tag="ps")
                for k in range(KFULL + 1):
                    st, sp = (k == 0), (k == KFULL)
                    for half in range(pair):
                        lhsT, r0, r1 = mm_operands(tiles[half], k)
                        prow = half * B
                        nc.tensor.matmul(ps[prow:prow + B, 0:NSPLIT], lhsT, r0,
                                         start=st, stop=sp)
                        nc.tensor.matmul(ps[prow:prow + B, NSPLIT:P], lhsT, r1,
                                         start=st, stop=sp)
                out = op.tile([pair * B, P], F32, name=f"out{c0}", tag="out")
                nc.vector.tensor_copy(out[:, 0:NSPLIT], ps[:, 0:NSPLIT])
                nc.scalar.copy(out[:, NSPLIT:P], ps[:, NSPLIT:P])
                nc.scalar.dma_start(
                    y[c0:c0 + pair].rearrange("c b p -> (c b) p"), out[:])

    nc.compile()
    return nc


def _get_module():
    if "nc" not in _CACHE:
        _CACHE["nc"] = _build_module()
    return _CACHE["nc"]


def _prep_inputs(x, W, b):
    wt = np.zeros((CPAD, SP1, P), dtype=np.float32)
    wt[:C, :S, :] = W.transpose(0, 2, 1)
    wt[:C, S, :] = b
    xt = np.zeros((CPAD, SP1, B), dtype=np.float32)
    xt[:C, :S, :] = x.transpose(2, 1, 0)
    xt[:, S, :] = 1.0
    in_maps = []
    for i in range(N_CORES):
        sl = slice(i * CL, (i + 1) * CL)
        in_maps.append({
            "wt": np.ascontiguousarray(wt[sl]),
            "xt": np.ascontiguousarray(xt[sl]),
        })
    return in_maps


def _gather(results):
    ys = np.concatenate([results[i]["y"] for i in range(N_CORES)], axis=0)
    return np.ascontiguousarray(ys[:C].transpose(1, 2, 0))


def run(x, W, b, **run_kwargs):
    """Full pipeline, returns (output, BassKernelResults)."""
    nc = _get_module()
    in_maps = _prep_inputs(np.asarray(x), np.asarray(W), np.asarray(b))
    res = run_bass_kernel_spmd(nc, in_maps, list(range(N_CORES)), **run_kwargs)
    return _gather(res.results), res


def kernel(x, W, b):
    out, _ = run(x, W, b)
    return out


# revision 6
# speedup vs baseline: 1.1278x; 1.1278x over previous
"""Per-channel Linear(seq->pred) over channels, 8-core channel-parallel Trainium2 kernel.

Math: y[b,p,c] = sum_s x[b,s,c] * W[c,p,s] + bias[c,p]

Strategy:
  - Shard channels C=321 across 8 cores (pad to 328 = 8*41).
  - Host-side re-layout:
      wt[c,s,p] = W[c,p,s], with an extra row wt[c,720,p] = bias[c,p]
      xt[c,s,b] = x[b,s,c], with an extra row xt[c,720,b] = 1.0
    so bias is folded into the contraction (K = 721).
  - Per channel: Y_c[b,p] = sum_k xT_chunk[k].T @ wT_chunk[k], accumulated in
    PSUM over 6 K-chunks (5x128 + 81). lhsT = xT chunk [K,64] (stationary),
    rhs = wT chunk [K,720] streamed as N = 512 + 208 (PSUM bank limit).
  - Two channels share one PSUM tile via PE column tiling: channel A in
    output partitions 0:64, channel B in 64:128 (concurrent col-groups).
  - Result copied PSUM->SBUF (DVE + ACT split) and DMA'd out as y[c,b,p].
"""

import numpy as np

import concourse.bacc as bacc
import concourse.mybir as mybir
import concourse.tile as tile
from concourse.bass_utils import run_bass_kernel_spmd

F32 = mybir.dt.float32

B = 64          # batch
S = 720         # seq_len (contraction)
P = 720         # pred_len
C = 321         # channels
N_CORES = 8
CL = 41         # channels per core; 8*41 = 328 >= 321
CPAD = N_CORES * CL
SP1 = S + 1     # contraction rows incl. bias row
KFULL = 5       # number of full 128-row K chunks
KTAIL = SP1 - KFULL * 128  # 81
NSPLIT = 512    # first matmul N (PSUM bank holds 512 f32)

_CACHE: dict = {}


def _build_module():
    nc = bacc.Bacc("TRN2", target_bir_lowering=False, debug=False,
                   num_devices=N_CORES)
    wt = nc.dram_tensor("wt", [CL, SP1, P], F32, kind="ExternalInput").ap()
    xt = nc.dram_tensor("xt", [CL, SP1, B], F32, kind="ExternalInput").ap()
    y = nc.dram_tensor("y", [CL, B, P], F32, kind="ExternalOutput").ap()

    with tile.TileContext(nc) as tc:
        with (
            tc.tile_pool(name="wp", bufs=3) as wp,
            tc.tile_pool(name="wtp", bufs=3) as wtp,
            tc.tile_pool(name="xp", bufs=3) as xp,
            tc.tile_pool(name="xtp", bufs=3) as xtp,
            tc.tile_pool(name="pp", bufs=3, space="PSUM") as pp,
            tc.tile_pool(name="op", bufs=3) as op,
        ):
            def load_pair(c0, pair):
                wbig = wp.tile([128, pair, KFULL, P], F32, name=f"wbig{c0}", tag="wbig")
                wlast = wtp.tile([128, pair, P], F32, name=f"wlast{c0}", tag="wlast")
                xbig = xp.tile([128, pair, KFULL, B], F32, name=f"xbig{c0}", tag="xbig")
                xlast = xtp.tile([128, pair, B], F32, name=f"xlast{c0}", tag="xlast")
                # DMA APs are limited to 3 dims after balancing, so load
                # per-channel (1.76 MB each) into the pair-shaped tiles.
                for j in range(pair):
                    c = c0 + j
                    nc.sync.dma_start(
                        wbig[:, j], wt[c, 0:KFULL * 128, :].rearrange("(k s) p -> s k p", s=128))
                    nc.sync.dma_start(wlast[:KTAIL, j], wt[c, KFULL * 128:SP1, :])
                    nc.sync.dma_start(
                        xbig[:, j], xt[c, 0:KFULL * 128, :].rearrange("(k s) b -> s k b", s=128))
                    nc.sync.dma_start(xlast[:KTAIL, j], xt[c, KFULL * 128:SP1, :])
                return wbig, wlast, xbig, xlast

            # process channels in pairs: two channels share one PSUM tile
            # (output partitions 0:64 and 64:128 -> PE column tiling).
            # Matmuls of the two channels are interleaved so the two
            # 64-wide column groups stream concurrently in the PE array.
            for c0 in range(0, CL, 2):
                pair = min(2, CL - c0)
                wbig, wlast, xbig, xlast = load_pair(c0, pair)
                ps = pp.tile([pair * B, P], F32, name=f"ps{c0}", tag="ps")
                for k in range(KFULL + 1):
                    st, sp = (k == 0), (k == KFULL)
                    for half in range(pair):
                        if k < KFULL:
                            lhsT = xbig[:, half, k, :]
                            r0 = wbig[:, half, k, 0:NSPLIT]
                            r1 = wbig[:, half, k, NSPLIT:P]
                        else:
                            lhsT = xlast[:KTAIL, half, :]
                            r0 = wlast[:KTAIL, half, 0:NSPLIT]
                            r1 = wlast[:KTAIL, half, NSPLIT:P]
                        prow = half * B
                        nc.tensor.matmul(ps[prow:prow + B, 0:NSPLIT], lhsT, r0,
                                         start=st, stop=sp)
                        nc.tensor.matmul(ps[prow:prow + B, NSPLIT:P], lhsT, r1,
                                         start=st, stop=sp)
                out = op.tile([pair * B, P], F32, name=f"out{c0}", tag="out")
                nc.vector.tensor_copy(out[:, 0:NSPLIT], ps[:, 0:NSPLIT])
                nc.scalar.copy(out[:, NSPLIT:P], ps[:, NSPLIT:P])
                nc.sync.dma_start(
                    y[c0:c0 + pair].rearrange("c b p -> (c b) p"), out[:])

    nc.compile()
    return nc


def _get_module():
    if "nc" not in _CACHE:
        _CACHE["nc"] = _build_module()
    return _CACHE["nc"]


def _prep_inputs(x, W, b):
    wt = np.zeros((CPAD, SP1, P), dtype=np.float32)
    wt[:C, :S, :] = W.transpose(0, 2, 1)
    wt[:C, S, :] = b
    xt = np.zeros((CPAD, SP1, B), dtype=np.float32)
    xt[:C, :S, :] = x.transpose(2, 1, 0)
    xt[:, S, :] = 1.0
    in_maps = []
    for i in range(N_CORES):
        sl = slice(i * CL, (i + 1) * CL)
        in_maps.append({
            "wt": np.ascontiguousarray(wt[sl]),
            "xt": np.ascontiguousarray(xt[sl]),
        })
    return in_maps


def _gather(results):
    ys = np.concatenate([results[i]["y"] for i in range(N_CORES)], axis=0)
    return np.ascontiguousarray(ys[:C].transpose(1, 2, 0))


def run(x, W, b, **run_kwargs):
    """Full pipeline, returns (output, BassKernelResults)."""
    nc = _get_module()
    in_maps = _prep_inputs(np.asarray(x), np.asarray(W), np.asarray(b))
    res = run_bass_kernel_spmd(nc, in_maps, list(range(N_CORES)), **run_kwargs)
    return _gather(res.results), res


def kernel(x, W, b):
    out, _ = run(x, W, b)
    return out
